# revision 18
# baseline (speedup 1.0000x reference)
"""TENER-style MultiHeadedAttention TRN2 kernel (8 NeuronCores, SPMD), v2.

Sharding: core c handles batch b = c//4 and query rows [256*(c%4), +256).
Host gather is pure concatenation.

Math (same trick as v1): TENER's relative-position term after the shift is
  rel[s, j] = (q_s + v_bias_h) . pos[S + j - s]
which by angle addition folds into ONE 128-deep contraction per head:
  scoresT[j, s] = [k_j ; sin(w j) ; cos(w j)] . [q_s ; a_sin(s) ; a_cos(s)]

v2 changes vs v1 (177us):
  * fp16 score path (Wq, query, catq, kg)  -> FWL-enabled matmuls + half DMA
  * bf16 value path (Wv, value, vv, ex, xn, Wo) -> half DMA, 2x DVE
  * exp on [128, 1024] PSUM tiles (amortizes ACT's 352-cycle op overhead)
  * xt copied PSUM->SBUF right after the attnV chain (frees PSUM banks,
    enables 2-deep score double-buffering without bank overflow)
  * normalization: denom row -> PE broadcast -> one DVE reciprocal per
    head-pair (v1 did a [64,256] reciprocal per head = 28us DVE)
  * g-table stored once; per-head kg tiles filled by SBUF->SBUF DMA
  * continuous PE stream from ~2us (HAM stays at K=8/8 warm clock)
"""

import math
import sys

sys.path.insert(0, "/opt/trn_rl_repo")

import numpy as np

B, S, D = 2, 1024, 1024
H, HD = 16, 64          # heads, head_dim
HALF = 32               # sin/cos half of head_dim
NC_ = 8                 # cores
SP = 256                # query rows per core
JT = S // 128           # 8 key tiles
FT = D // 128           # 8 feature tiles

_cache: dict = {}


def _build_nc():
    import concourse.bacc as bacc
    import concourse.mybir as mybir
    from concourse import tile

    F32 = mybir.dt.float32
    F16 = mybir.dt.float16
    BF16 = mybir.dt.bfloat16
    ADD = mybir.AluOpType.add
    SUB = mybir.AluOpType.subtract
    MUL = mybir.AluOpType.mult
    EXP = mybir.ActivationFunctionType.Exp

    import os
    DBG = int(os.environ.get("BASS_KERNEL_DEBUG", "0"))

    nc = bacc.Bacc("TRN2", target_bir_lowering=False, debug=False, num_devices=NC_)

    # DRAM inputs
    qpk_d = nc.dram_tensor("qpk", [D + 1, D + SP], F16, kind="ExternalInput")
    vpk_d = nc.dram_tensor("vpk", [D, S], BF16, kind="ExternalInput")
    wvt_d = nc.dram_tensor("wvt", [D, D], BF16, kind="ExternalInput")
    kgp_d = nc.dram_tensor("kgp", [D, S], F16, kind="ExternalInput")
    gt_d = nc.dram_tensor("gt", [64, S], F16, kind="ExternalInput")
    wot_d = nc.dram_tensor("wot", [D + 1, D], BF16, kind="ExternalInput")
    tabs_d = nc.dram_tensor("tabs", [128, 2 * SP], F16, kind="ExternalInput")
    vbt_d = nc.dram_tensor("vbt", [128, FT], F32, kind="ExternalInput")
    out_d = nc.dram_tensor("out", [SP, D], F32, kind="ExternalOutput")
    if DBG:
        dbg_catq_d = nc.dram_tensor("dbg_catq", [128, SP], F16, kind="ExternalOutput")
        dbg_kg_d = nc.dram_tensor("dbg_kg", [128, S], F16, kind="ExternalOutput")
        dbg_vv_d = nc.dram_tensor("dbg_vv", [128, H * 66], BF16, kind="ExternalOutput")
        dbg_xts_d = nc.dram_tensor("dbg_xts", [65, 512], F32, kind="ExternalOutput")
        dbg_ex_d = nc.dram_tensor("dbg_ex", [128, 1024], BF16, kind="ExternalOutput")
        dbg_xn_d = nc.dram_tensor("dbg_xn", [128, SP], BF16, kind="ExternalOutput")

    with tile.TileContext(nc, num_cores=NC_) as tc:
        with tc.tile_pool(name="persist", bufs=1) as pp, \
             tc.tile_pool(name="small", bufs=3) as sp, \
             tc.tile_pool(name="expool", bufs=4) as ep:

            # ---- persistent SBUF tiles ----
            tabs = pp.tile([128, 2 * SP], F16, tag="tabs")      # cos_s | sin_s
            vbt = pp.tile([128, FT], F32, tag="vbt")
            gt = pp.tile([64, S], F16, tag="gt")
            nc.sync.dma_start(tabs[:], tabs_d.ap())
            nc.sync.dma_start(vbt[:], vbt_d.ap())
            nc.sync.dma_start(gt[:], gt_d.ap())

            catq = [pp.tile([128, SP], F16, name=f"catq{h}", tag=f"catq{h}") for h in range(H)]
            kg = [pp.tile([128, S], F16, name=f"kg{h}", tag=f"kg{h}") for h in range(H)]
            vv = [pp.tile([128, H * 66], BF16, name=f"vv{j}", tag=f"vv{j}") for j in range(JT)]
            # xn[i]: out-proj stationary for heads (2i, 2i+1): [128 cdims, 256 q]
            xn = [pp.tile([128, SP], BF16, name=f"xn{i}", tag=f"xn{i}") for i in range(FT)]

            ebias = pp.tile([128, 1], F32, tag="ebias")
            nc.vector.memset(ebias[:], -25.0)
            ones1 = pp.tile([1, 128], BF16, tag="ones1")        # bcast / bias row
            nc.vector.memset(ones1[:], 1.0)
            onesc = pp.tile([128, 16], BF16, tag="onesc")       # vv ones cols
            nc.vector.memset(onesc[:], 1.0)

            # ---------- phase 1: q projection + rotation ----------
            with tc.tile_pool(name="qpk", bufs=1) as qpkp, \
                 tc.tile_pool(name="qps", bufs=2, space="PSUM") as qps:
                qpk = []
                for c in range(FT):
                    t = qpkp.tile([128, D + SP], F16, tag=f"qpk{c}")
                    eng = nc.sync if c % 2 == 0 else nc.gpsimd
                    eng.dma_start(t[:], qpk_d.ap()[c * 128:(c + 1) * 128, :])
                    qpk.append(t)
                qpk9 = qpkp.tile([1, D + SP], F16, tag="qpk9")
                nc.sync.dma_start(qpk9[:], qpk_d.ap()[D:D + 1, :])

                for ft in range(FT):
                    qpsum = qps.tile([128, SP], F32, tag="qpsum")
                    for c in range(FT):
                        nc.tensor.matmul(
                            qpsum[:], qpk[c][:, ft * 128:(ft + 1) * 128],
                            qpk[c][:, D:D + SP], start=(c == 0), stop=False)
                    nc.tensor.matmul(qpsum[:], qpk9[:, ft * 128:(ft + 1) * 128],
                                     qpk9[:, D:D + SP], start=False, stop=True)

                    # q halves into catq rows 0:64 (partition-shift copies)
                    nc.vector.tensor_copy(catq[2 * ft][0:64, :], qpsum[0:64, :])
                    nc.vector.tensor_copy(catq[2 * ft + 1][0:64, :], qpsum[64:128, :])

                    # rotation -> catq rows 64:128
                    qv = sp.tile([128, SP], F16, tag="qv")
                    nc.vector.tensor_scalar(
                        out=qv[:], in0=qpsum[:],
                        scalar1=vbt[:, ft:ft + 1], scalar2=None, op0=ADD)
                    t1 = sp.tile([128, SP], F16, tag="t1")
                    nc.vector.tensor_tensor(out=t1[:], in0=qv[:],
                                            in1=tabs[:, 0:SP], op=MUL)
                    t2 = sp.tile([128, SP], F16, tag="t2")
                    for g in range(4):
                        src = [32, 0, 96, 64][g]
                        nc.vector.tensor_tensor(
                            out=t2[g * 32:(g + 1) * 32, :],
                            in0=qv[src:src + 32, :],
                            in1=tabs[src:src + 32, SP:2 * SP], op=MUL)
                    for par in range(2):
                        hq = 2 * ft + par
                        o_ = par * 64
                        nc.vector.tensor_tensor(
                            out=catq[hq][64:96, :], in0=t1[o_:o_ + 32, :],
                            in1=t2[o_:o_ + 32, :], op=ADD)
                        nc.vector.tensor_tensor(
                            out=catq[hq][96:128, :], in0=t1[o_ + 32:o_ + 64, :],
                            in1=t2[o_ + 32:o_ + 64, :], op=SUB)

            # ---------- phase 2: v projection ----------
            # stationary value.T tiles, moving Wv.T -> psum [128 pos, 512 out]
            with tc.tile_pool(name="wvp", bufs=1) as wvpp, \
                 tc.tile_pool(name="vps", bufs=3, space="PSUM") as vps:
                wvp = []
                valt = []
                for c in range(FT):
                    t = wvpp.tile([128, D], BF16, tag=f"wvp{c}")
                    nc.gpsimd.dma_start(t[:], wvt_d.ap()[c * 128:(c + 1) * 128, :])
                    wvp.append(t)
                    t2_ = wvpp.tile([128, S], BF16, tag=f"valt{c}")
                    nc.sync.dma_start(t2_[:], vpk_d.ap()[c * 128:(c + 1) * 128, :])
                    valt.append(t2_)

                # kg DMAs issued here; consumed in phase 3
                for h in range(H):
                    eng = nc.sync if h % 2 == 0 else nc.gpsimd
                    eng.dma_start(kg[h][0:64, :],
                                  kgp_d.ap()[h * 64:(h + 1) * 64, :])
                    eng.dma_start(kg[h][64:128, :], gt[:])

                for jt in range(JT):
                    # ones columns (denominator trick)
                    nc.scalar.copy(
                        vv[jt][:].rearrange("p (h x) -> p h x", x=66)[:, :, 64:65],
                        onesc[:].rearrange("p (h x) -> p h x", x=1))
                    for hf in range(2):
                        vpsum = vps.tile([128, 512], F32, tag="vpsum")
                        for c in range(FT):
                            nc.tensor.matmul(
                                vpsum[:],
                                valt[c][:, jt * 128:(jt + 1) * 128],
                                wvp[c][:, hf * 512:(hf + 1) * 512],
                                start=(c == 0), stop=(c == FT - 1))
                        dst = vv[jt][:, hf * 528:(hf + 1) * 528].rearrange(
                            "p (h x) -> p h x", x=66)[:, :, 0:64]
                        src_ = vpsum[:].rearrange("p (h d) -> p h d", d=64)
                        nc.scalar.copy(dst, src_)

            # wot DMA early (phase 4 input)
            with tc.tile_pool(name="wop", bufs=1) as wop:
                wo = []
                for c in range(FT):
                    t = wop.tile([128, D], BF16, tag=f"wo{c}")
                    nc.gpsimd.dma_start(t[:], wot_d.ap()[c * 128:(c + 1) * 128, :])
                    wo.append(t)
                wo9 = wop.tile([1, D], BF16, tag="wo9")
                nc.sync.dma_start(wo9[:], wot_d.ap()[D:D + 1, :])

                # ---------- phase 3: attention (4 heads per quad) ----------
                with tc.tile_pool(name="scps", bufs=2, space="PSUM") as scps, \
                     tc.tile_pool(name="xtps", bufs=1, space="PSUM") as xtps, \
                     tc.tile_pool(name="rbps", bufs=2, space="PSUM") as rbps:
                    for quad in range(4):
                        hs = [4 * quad + i for i in range(4)]
                        # xt pair tiles: [65, 512] = 2 heads x 256 q
                        xt = [xtps.tile([66, 512], F32, name=f"xt{p}", tag=f"xt{p}")
                              for p in range(2)]
                        scs = []
                        exs = []
                        for jt in range(JT + 1):
                            if jt < JT:
                                sc = scps.tile([128, 1024], F32, tag="sc")
                                for i, h in enumerate(hs):
                                    nc.tensor.matmul(
                                        sc[:, i * 256:(i + 1) * 256],
                                        kg[h][:, jt * 128:(jt + 1) * 128],
                                        catq[h][:], start=True, stop=True,
                                        skip_group_check=True)
                                ex = ep.tile([128, 1024], BF16, tag="ex")
                                nc.scalar.activation(ex[:], sc[:], EXP,
                                                     bias=ebias[:], scale=1.0)
                                if DBG and quad == 0 and jt == 0:
                                    nc.sync.dma_start(dbg_ex_d.ap(), ex[:])
                                scs.append(sc)
                                exs.append(ex)
                            if jt >= 1:
                                jp = jt - 1
                                exp_ = exs[jp]
                                for i, h in enumerate(hs):
                                    # start=True clears the WHOLE psum bank, so
                                    # only the bank's first chain (even i) may
                                    # start; the odd chain overwrites via the
                                    # cleared has_written bits.
                                    nc.tensor.matmul(
                                        xt[i // 2][:, (i % 2) * 256:(i % 2) * 256 + 256],
                                        vv[jp][:, h * 66:h * 66 + 66],
                                        exp_[:, i * 256:(i + 1) * 256],
                                        start=(jp == 0 and i % 2 == 0),
                                        stop=(jp == JT - 1),
                                        skip_group_check=True)

                        # normalization per head-pair
                        for p in range(2):
                            xts = sp.tile([65, 512], F32, tag="xts")
                            nc.vector.tensor_copy(xts[:], xt[p][0:65, :])
                            if DBG and quad == 0 and p == 0:
                                nc.sync.dma_start(dbg_xts_d.ap(), xts[:])
                            den2 = sp.tile([1, 512], BF16, tag="den2")
                            nc.vector.tensor_copy(den2[0:1, 0:256],
                                                  xts[64:65, 0:256])
                            nc.vector.tensor_copy(den2[0:1, 256:512],
                                                  xts[64:65, 256:512])
                            rb = rbps.tile([128, 512], F32, tag="rb")
                            nc.tensor.matmul(rb[:], ones1[:], den2[:],
                                             start=True, stop=True,
                                             skip_group_check=True)
                            rrec = sp.tile([128, 512], F32, tag="rrec")
                            nc.vector.reciprocal_approx_fast(rrec[:], rb[:])
                            xni = 2 * quad + p
                            nc.vector.tensor_tensor(
                                out=xn[xni][0:64, :], in0=xts[0:64, 0:256],
                                in1=rrec[0:64, 0:256], op=MUL)
                            nc.vector.tensor_tensor(
                                out=xn[xni][64:128, :], in0=xts[0:64, 256:512],
                                in1=rrec[0:64, 256:512], op=MUL)

                # ---------- phase 4: output projection ----------
                with tc.tile_pool(name="ops", bufs=2, space="PSUM") as ops, \
                     tc.tile_pool(name="osb", bufs=2) as osb:
                    for st in range(2):
                        for hf in range(2):
                            op = ops.tile([128, 512], F32, tag="op")
                            for c in range(FT):
                                nc.tensor.matmul(
                                    op[:], xn[c][:, st * 128:(st + 1) * 128],
                                    wo[c][:, hf * 512:(hf + 1) * 512],
                                    start=(c == 0), stop=False)
                            nc.tensor.matmul(
                                op[:], ones1[:],
                                wo9[:, hf * 512:(hf + 1) * 512],
                                start=False, stop=True)
                            os_ = osb.tile([128, 512], F32, tag="os")
                            nc.scalar.copy(os_[:], op[:])
                            nc.sync.dma_start(
                                out_d.ap()[st * 128:(st + 1) * 128,
                                           hf * 512:(hf + 1) * 512], os_[:])
                    if DBG:
                        nc.sync.dma_start(dbg_catq_d.ap(), catq[0][:])
                        nc.sync.dma_start(dbg_kg_d.ap(), kg[0][:])
                        nc.sync.dma_start(dbg_vv_d.ap(), vv[0][:])
                        nc.sync.dma_start(dbg_xn_d.ap(), xn[0][:])

    nc.finalize()
    return nc


def _host_pack(query, key, value, Wq, bq, Wv, bv, Wo, bo, v_bias):
    """Build the 8 per-core input maps."""
    import ml_dtypes
    bf16 = ml_dtypes.bfloat16
    w = np.exp(np.arange(HALF) * (-math.log(10000.0) / (HALF - 1))).astype(np.float64)

    WqT = np.concatenate([Wq.T, bq[None, :]], axis=0)          # [1025, 1024]
    bo_eff = bo + Wo @ bv                                      # bv folds out (softmax sums to 1)
    wot = np.concatenate([Wo.T, bo_eff[None, :]], axis=0).astype(bf16)
    wvt = Wv.T.astype(bf16)

    # g table [64, S]: rows 0:32 sin(w j), 32:64 cos(w j)
    j = np.arange(S, dtype=np.float64)
    gsin = np.sin(w[:, None] * j[None, :])
    gcos = np.cos(w[:, None] * j[None, :])
    gt = np.concatenate([gsin, gcos], axis=0).astype(np.float16)

    kgps = [key[b].T.astype(np.float16) for b in range(B)]
    vpks = [value[b].T.astype(bf16) for b in range(B)]

    vbflat = v_bias.reshape(-1).astype(np.float32)             # [1024] (h,dh)
    vbt = vbflat.reshape(FT, 128).T.copy()                     # [128, 8]

    in_maps = []
    for c in range(NC_):
        b, sl = c // 4, c % 4
        s0 = sl * SP
        qp = np.empty((D + 1, D + SP), np.float32)
        qp[:D, :D] = WqT[:D]
        qp[:D, D:] = query[b].T[:, s0:s0 + SP]
        qp[D, :D] = WqT[D]
        qp[D, D:] = 1.0

        svals = (s0 + np.arange(SP, dtype=np.float64))[None, :]  # [1, 256]
        wrep = np.tile(w, 4)[:, None]                            # [128, 1]
        tabs = np.empty((128, 2 * SP), np.float32)
        tabs[:, 0:SP] = np.cos(wrep * svals)
        tabs[:, SP:2 * SP] = np.sin(wrep * svals)

        in_maps.append({
            "qpk": qp.astype(np.float16),
            "vpk": vpks[b],
            "wvt": wvt,
            "kgp": kgps[b],
            "gt": gt,
            "wot": wot,
            "tabs": tabs.astype(np.float16),
            "vbt": vbt,
        })
    return in_maps


def kernel(query, key, value, mask, Wq, bq, Wv, bv, Wo, bo, v_bias):
    from concourse.bass_utils import run_bass_kernel_spmd

    query = np.asarray(query, np.float32)
    key = np.asarray(key, np.float32)
    value = np.asarray(value, np.float32)
    in_maps = _host_pack(query, key, value,
                         np.asarray(Wq, np.float32), np.asarray(bq, np.float32),
                         np.asarray(Wv, np.float32), np.asarray(bv, np.float32),
                         np.asarray(Wo, np.float32), np.asarray(bo, np.float32),
                         np.asarray(v_bias, np.float32))

    if "nc" not in _cache:
        _cache["nc"] = _build_nc()
    nc = _cache["nc"]

    import os
    if int(os.environ.get("BASS_KERNEL_TRACE", "0")):
        if "antenv.axon_hooks" not in sys.modules:
            import types
            import antenv
            _mod = types.ModuleType("antenv.axon_hooks")
            _box = [None]
            _mod.set_axon_ntff_profile_hook = lambda h: _box.__setitem__(0, h)
            _mod.get_axon_ntff_profile_hook = lambda: _box[0]
            sys.modules["antenv.axon_hooks"] = _mod
            antenv.axon_hooks = _mod
            if "/root/.axon_site" not in sys.path:
                sys.path.insert(0, "/root/.axon_site")
            from trn_agent_boot.trn_boot import _ntff_profile_via_ctypes
            _mod.set_axon_ntff_profile_hook(
                _ntff_profile_via_ctypes("/opt/axon/libaxon_pjrt.so"))
    res = run_bass_kernel_spmd(
        nc, in_maps, core_ids=list(range(NC_)),
        trace=bool(int(os.environ.get("BASS_KERNEL_TRACE", "0"))))
    _cache["last_result"] = res

    out = np.empty((B, S, D), np.float32)
    for c in range(NC_):
        b, sl = c // 4, c % 4
        out[b, sl * SP:(sl + 1) * SP, :] = np.asarray(
            res.results[c]["out"], np.float32)
    return out


# revision 20
# speedup vs baseline: 1.0765x; 1.0765x over previous
"""TENER-style MultiHeadedAttention TRN2 kernel (8 NeuronCores, SPMD), v2.

Sharding: core c handles batch b = c//4 and query rows [256*(c%4), +256).
Host gather is pure concatenation.

Math (same trick as v1): TENER's relative-position term after the shift is
  rel[s, j] = (q_s + v_bias_h) . pos[S + j - s]
which by angle addition folds into ONE 128-deep contraction per head:
  scoresT[j, s] = [k_j ; sin(w j) ; cos(w j)] . [q_s ; a_sin(s) ; a_cos(s)]

v2 changes vs v1 (177us):
  * fp16 score path (Wq, query, catq, kg)  -> FWL-enabled matmuls + half DMA
  * bf16 value path (Wv, value, vv, ex, xn, Wo) -> half DMA, 2x DVE
  * exp on [128, 1024] PSUM tiles (amortizes ACT's 352-cycle op overhead)
  * xt copied PSUM->SBUF right after the attnV chain (frees PSUM banks,
    enables 2-deep score double-buffering without bank overflow)
  * normalization: denom row -> PE broadcast -> one DVE reciprocal per
    head-pair (v1 did a [64,256] reciprocal per head = 28us DVE)
  * g-table stored once; per-head kg tiles filled by SBUF->SBUF DMA
  * continuous PE stream from ~2us (HAM stays at K=8/8 warm clock)
"""

import math
import sys

sys.path.insert(0, "/opt/trn_rl_repo")

import numpy as np

B, S, D = 2, 1024, 1024
H, HD = 16, 64          # heads, head_dim
HALF = 32               # sin/cos half of head_dim
NC_ = 8                 # cores
SP = 256                # query rows per core
JT = S // 128           # 8 key tiles
FT = D // 128           # 8 feature tiles

_cache: dict = {}


def _build_nc():
    import concourse.bacc as bacc
    import concourse.mybir as mybir
    from concourse import tile

    F32 = mybir.dt.float32
    F16 = mybir.dt.float16
    BF16 = mybir.dt.bfloat16
    ADD = mybir.AluOpType.add
    SUB = mybir.AluOpType.subtract
    MUL = mybir.AluOpType.mult
    EXP = mybir.ActivationFunctionType.Exp

    import os
    DBG = int(os.environ.get("BASS_KERNEL_DEBUG", "0"))

    nc = bacc.Bacc("TRN2", target_bir_lowering=False, debug=False, num_devices=NC_)

    # DRAM inputs
    qpk_d = nc.dram_tensor("qpk", [D + 1, D + SP], F16, kind="ExternalInput")
    vpk_d = nc.dram_tensor("vpk", [D, S], BF16, kind="ExternalInput")
    wvt_d = nc.dram_tensor("wvt", [D, D], BF16, kind="ExternalInput")
    kgp_d = nc.dram_tensor("kgp", [D, S], F16, kind="ExternalInput")
    gt_d = nc.dram_tensor("gt", [64, S], F16, kind="ExternalInput")
    wot_d = nc.dram_tensor("wot", [D + 1, D], BF16, kind="ExternalInput")
    tabs_d = nc.dram_tensor("tabs", [128, 2 * SP], F16, kind="ExternalInput")
    vbt_d = nc.dram_tensor("vbt", [128, FT], F32, kind="ExternalInput")
    out_d = nc.dram_tensor("out", [SP, D], F32, kind="ExternalOutput")
    if DBG:
        dbg_catq_d = nc.dram_tensor("dbg_catq", [128, SP], F16, kind="ExternalOutput")
        dbg_kg_d = nc.dram_tensor("dbg_kg", [128, S], F16, kind="ExternalOutput")
        dbg_vv_d = nc.dram_tensor("dbg_vv", [128, H * 66], BF16, kind="ExternalOutput")
        dbg_xts_d = nc.dram_tensor("dbg_xts", [65, 512], F32, kind="ExternalOutput")
        dbg_ex_d = nc.dram_tensor("dbg_ex", [128, 1024], BF16, kind="ExternalOutput")
        dbg_xn_d = nc.dram_tensor("dbg_xn", [128, SP], BF16, kind="ExternalOutput")

    with tile.TileContext(nc, num_cores=NC_) as tc:
        with tc.tile_pool(name="persist", bufs=1) as pp, \
             tc.tile_pool(name="small", bufs=3) as sp, \
             tc.tile_pool(name="expool", bufs=4) as ep:

            # ---- persistent SBUF tiles ----
            tabs = pp.tile([128, 2 * SP], F16, tag="tabs")      # cos_s | sin_s
            vbt = pp.tile([128, FT], F32, tag="vbt")
            gt = pp.tile([64, S], F16, tag="gt")
            nc.sync.dma_start(tabs[:], tabs_d.ap())
            nc.sync.dma_start(vbt[:], vbt_d.ap())
            nc.sync.dma_start(gt[:], gt_d.ap())

            catq = [pp.tile([128, SP], F16, name=f"catq{h}", tag=f"catq{h}") for h in range(H)]
            kg = [pp.tile([128, S], F16, name=f"kg{h}", tag=f"kg{h}") for h in range(H)]
            vv = [pp.tile([128, H * 66], BF16, name=f"vv{j}", tag=f"vv{j}") for j in range(JT)]
            # xn[i]: out-proj stationary for heads (2i, 2i+1): [128 cdims, 256 q]
            xn = [pp.tile([128, SP], BF16, name=f"xn{i}", tag=f"xn{i}") for i in range(FT)]

            ebias = pp.tile([128, 1], F32, tag="ebias")
            nc.vector.memset(ebias[:], -25.0)
            ones1 = pp.tile([1, 128], BF16, tag="ones1")        # bcast / bias row
            nc.vector.memset(ones1[:], 1.0)
            onesc = pp.tile([128, 16], BF16, tag="onesc")       # vv ones cols
            nc.vector.memset(onesc[:], 1.0)

            # ---------- phase 1: q projection + rotation ----------
            with tc.tile_pool(name="qpk", bufs=1) as qpkp, \
                 tc.tile_pool(name="qps", bufs=1, space="PSUM") as qps:
                qpk = []
                for c in range(FT):
                    t = qpkp.tile([128, D + SP], F16, tag=f"qpk{c}")
                    eng = nc.sync if c % 2 == 0 else nc.gpsimd
                    eng.dma_start(t[:], qpk_d.ap()[c * 128:(c + 1) * 128, :])
                    qpk.append(t)
                qpk9 = qpkp.tile([1, D + SP], F16, tag="qpk9")
                nc.sync.dma_start(qpk9[:], qpk_d.ap()[D:D + 1, :])

                # contraction-outer: 8 interleaved psum chains so matmuls run
                # as qpk tiles arrive from DMA (keeps PE dense + HAM warm).
                # Chains ft, ft+1 share a psum bank; only the even chain may
                # issue start=True (bank-clear), odd relies on has_written.
                qtile = [qps.tile([128, 512], F32, name=f"qt{i}", tag=f"qt{i}")
                         for i in range(4)]

                def qsum(ft):
                    return qtile[ft // 2][:, (ft % 2) * SP:(ft % 2) * SP + SP]

                for c in range(FT):
                    for ft in range(FT):
                        nc.tensor.matmul(
                            qsum(ft), qpk[c][:, ft * 128:(ft + 1) * 128],
                            qpk[c][:, D:D + SP],
                            start=(c == 0 and ft % 2 == 0), stop=False,
                            skip_group_check=True)
                for ft in range(FT):
                    nc.tensor.matmul(qsum(ft), qpk9[:, ft * 128:(ft + 1) * 128],
                                     qpk9[:, D:D + SP], start=False, stop=True,
                                     skip_group_check=True)

                for ft in range(FT):
                    qpsum = qsum(ft)
                    # q halves into catq rows 0:64 (partition-shift copies, ACT)
                    nc.scalar.copy(catq[2 * ft][0:64, :], qpsum[0:64, :])
                    nc.scalar.copy(catq[2 * ft + 1][0:64, :], qpsum[64:128, :])

                    # rotation -> catq rows 64:128
                    qv = sp.tile([128, SP], F16, tag="qv")
                    nc.vector.tensor_scalar(
                        out=qv[:], in0=qpsum[:],
                        scalar1=vbt[:, ft:ft + 1], scalar2=None, op0=ADD)
                    t1 = sp.tile([128, SP], F16, tag="t1")
                    nc.vector.tensor_tensor(out=t1[:], in0=qv[:],
                                            in1=tabs[:, 0:SP], op=MUL)
                    t2 = sp.tile([128, SP], F16, tag="t2")
                    for g in range(4):
                        src = [32, 0, 96, 64][g]
                        nc.vector.tensor_tensor(
                            out=t2[g * 32:(g + 1) * 32, :],
                            in0=qv[src:src + 32, :],
                            in1=tabs[src:src + 32, SP:2 * SP], op=MUL)
                    for par in range(2):
                        hq = 2 * ft + par
                        o_ = par * 64
                        nc.vector.tensor_tensor(
                            out=catq[hq][64:96, :], in0=t1[o_:o_ + 32, :],
                            in1=t2[o_:o_ + 32, :], op=ADD)
                        nc.vector.tensor_tensor(
                            out=catq[hq][96:128, :], in0=t1[o_ + 32:o_ + 64, :],
                            in1=t2[o_ + 32:o_ + 64, :], op=SUB)

            # ---------- phase 2: v projection ----------
            # stationary value.T tiles, moving Wv.T -> psum [128 pos, 512 out]
            with tc.tile_pool(name="wvp", bufs=1) as wvpp, \
                 tc.tile_pool(name="vps", bufs=3, space="PSUM") as vps:
                wvp = []
                valt = []
                for c in range(FT):
                    t = wvpp.tile([128, D], BF16, tag=f"wvp{c}")
                    nc.gpsimd.dma_start(t[:], wvt_d.ap()[c * 128:(c + 1) * 128, :])
                    wvp.append(t)
                    t2_ = wvpp.tile([128, S], BF16, tag=f"valt{c}")
                    nc.sync.dma_start(t2_[:], vpk_d.ap()[c * 128:(c + 1) * 128, :])
                    valt.append(t2_)

                # kg DMAs issued here; consumed in phase 3
                for h in range(H):
                    eng = nc.sync if h % 2 == 0 else nc.gpsimd
                    eng.dma_start(kg[h][0:64, :],
                                  kgp_d.ap()[h * 64:(h + 1) * 64, :])
                    eng.dma_start(kg[h][64:128, :], gt[:])

                for jt in range(JT):
                    # ones columns (denominator trick)
                    nc.scalar.copy(
                        vv[jt][:].rearrange("p (h x) -> p h x", x=66)[:, :, 64:65],
                        onesc[:].rearrange("p (h x) -> p h x", x=1))
                    for hf in range(2):
                        vpsum = vps.tile([128, 512], F32, tag="vpsum")
                        for c in range(FT):
                            nc.tensor.matmul(
                                vpsum[:],
                                valt[c][:, jt * 128:(jt + 1) * 128],
                                wvp[c][:, hf * 512:(hf + 1) * 512],
                                start=(c == 0), stop=(c == FT - 1))
                        dst = vv[jt][:, hf * 528:(hf + 1) * 528].rearrange(
                            "p (h x) -> p h x", x=66)[:, :, 0:64]
                        src_ = vpsum[:].rearrange("p (h d) -> p h d", d=64)
                        nc.scalar.copy(dst, src_)

            # wot DMA early (phase 4 input)
            with tc.tile_pool(name="wop", bufs=1) as wop:
                wo = []
                for c in range(FT):
                    t = wop.tile([128, D], BF16, tag=f"wo{c}")
                    nc.gpsimd.dma_start(t[:], wot_d.ap()[c * 128:(c + 1) * 128, :])
                    wo.append(t)
                wo9 = wop.tile([1, D], BF16, tag="wo9")
                nc.sync.dma_start(wo9[:], wot_d.ap()[D:D + 1, :])

                # ---------- phase 3: attention (4 heads per quad) ----------
                with tc.tile_pool(name="scps", bufs=2, space="PSUM") as scps, \
                     tc.tile_pool(name="xtps", bufs=1, space="PSUM") as xtps, \
                     tc.tile_pool(name="rbps", bufs=2, space="PSUM") as rbps:
                    for quad in range(4):
                        hs = [4 * quad + i for i in range(4)]
                        # xt pair tiles: [65, 512] = 2 heads x 256 q
                        xt = [xtps.tile([66, 512], F32, name=f"xt{p}", tag=f"xt{p}")
                              for p in range(2)]
                        scs = []
                        exs = []
                        for jt in range(JT + 1):
                            if jt < JT:
                                sc = scps.tile([128, 1024], F32, tag="sc")
                                for i, h in enumerate(hs):
                                    nc.tensor.matmul(
                                        sc[:, i * 256:(i + 1) * 256],
                                        kg[h][:, jt * 128:(jt + 1) * 128],
                                        catq[h][:], start=True, stop=True,
                                        skip_group_check=True)
                                ex = ep.tile([128, 1024], BF16, tag="ex")
                                nc.scalar.activation(ex[:], sc[:], EXP,
                                                     bias=ebias[:], scale=1.0)
                                if DBG and quad == 0 and jt == 0:
                                    nc.sync.dma_start(dbg_ex_d.ap(), ex[:])
                                scs.append(sc)
                                exs.append(ex)
                            if jt >= 1:
                                jp = jt - 1
                                exp_ = exs[jp]
                                for i, h in enumerate(hs):
                                    # start=True clears the WHOLE psum bank, so
                                    # only the bank's first chain (even i) may
                                    # start; the odd chain overwrites via the
                                    # cleared has_written bits.
                                    nc.tensor.matmul(
                                        xt[i // 2][:, (i % 2) * 256:(i % 2) * 256 + 256],
                                        vv[jp][:, h * 66:h * 66 + 66],
                                        exp_[:, i * 256:(i + 1) * 256],
                                        start=(jp == 0 and i % 2 == 0),
                                        stop=(jp == JT - 1),
                                        skip_group_check=True)

                        # normalization per head-pair
                        for p in range(2):
                            xts = sp.tile([65, 512], F32, tag="xts")
                            nc.vector.tensor_copy(xts[:], xt[p][0:65, :])
                            if DBG and quad == 0 and p == 0:
                                nc.sync.dma_start(dbg_xts_d.ap(), xts[:])
                            den2 = sp.tile([1, 512], BF16, tag="den2")
                            nc.vector.tensor_copy(den2[0:1, 0:256],
                                                  xts[64:65, 0:256])
                            nc.vector.tensor_copy(den2[0:1, 256:512],
                                                  xts[64:65, 256:512])
                            rb = rbps.tile([128, 512], F32, tag="rb")
                            nc.tensor.matmul(rb[:], ones1[:], den2[:],
                                             start=True, stop=True,
                                             skip_group_check=True)
                            rrec = sp.tile([128, 512], F32, tag="rrec")
                            nc.vector.reciprocal_approx_fast(rrec[:], rb[:])
                            xni = 2 * quad + p
                            nc.vector.tensor_tensor(
                                out=xn[xni][0:64, :], in0=xts[0:64, 0:256],
                                in1=rrec[0:64, 0:256], op=MUL)
                            nc.vector.tensor_tensor(
                                out=xn[xni][64:128, :], in0=xts[0:64, 256:512],
                                in1=rrec[0:64, 256:512], op=MUL)

                # ---------- phase 4: output projection ----------
                with tc.tile_pool(name="ops", bufs=2, space="PSUM") as ops, \
                     tc.tile_pool(name="osb", bufs=2) as osb:
                    for st in range(2):
                        for hf in range(2):
                            op = ops.tile([128, 512], F32, tag="op")
                            for c in range(FT):
                                nc.tensor.matmul(
                                    op[:], xn[c][:, st * 128:(st + 1) * 128],
                                    wo[c][:, hf * 512:(hf + 1) * 512],
                                    start=(c == 0), stop=False)
                            nc.tensor.matmul(
                                op[:], ones1[:],
                                wo9[:, hf * 512:(hf + 1) * 512],
                                start=False, stop=True)
                            os_ = osb.tile([128, 512], F32, tag="os")
                            nc.scalar.copy(os_[:], op[:])
                            nc.sync.dma_start(
                                out_d.ap()[st * 128:(st + 1) * 128,
                                           hf * 512:(hf + 1) * 512], os_[:])
                    if DBG:
                        nc.sync.dma_start(dbg_catq_d.ap(), catq[0][:])
                        nc.sync.dma_start(dbg_kg_d.ap(), kg[0][:])
                        nc.sync.dma_start(dbg_vv_d.ap(), vv[0][:])
                        nc.sync.dma_start(dbg_xn_d.ap(), xn[0][:])

    nc.finalize()
    return nc


def _host_pack(query, key, value, Wq, bq, Wv, bv, Wo, bo, v_bias):
    """Build the 8 per-core input maps."""
    import ml_dtypes
    bf16 = ml_dtypes.bfloat16
    w = np.exp(np.arange(HALF) * (-math.log(10000.0) / (HALF - 1))).astype(np.float64)

    WqT = np.concatenate([Wq.T, bq[None, :]], axis=0)          # [1025, 1024]
    bo_eff = bo + Wo @ bv                                      # bv folds out (softmax sums to 1)
    wot = np.concatenate([Wo.T, bo_eff[None, :]], axis=0).astype(bf16)
    wvt = Wv.T.astype(bf16)

    # g table [64, S]: rows 0:32 sin(w j), 32:64 cos(w j)
    j = np.arange(S, dtype=np.float64)
    gsin = np.sin(w[:, None] * j[None, :])
    gcos = np.cos(w[:, None] * j[None, :])
    gt = np.concatenate([gsin, gcos], axis=0).astype(np.float16)

    kgps = [key[b].T.astype(np.float16) for b in range(B)]
    vpks = [value[b].T.astype(bf16) for b in range(B)]

    vbflat = v_bias.reshape(-1).astype(np.float32)             # [1024] (h,dh)
    vbt = vbflat.reshape(FT, 128).T.copy()                     # [128, 8]

    in_maps = []
    for c in range(NC_):
        b, sl = c // 4, c % 4
        s0 = sl * SP
        qp = np.empty((D + 1, D + SP), np.float32)
        qp[:D, :D] = WqT[:D]
        qp[:D, D:] = query[b].T[:, s0:s0 + SP]
        qp[D, :D] = WqT[D]
        qp[D, D:] = 1.0

        svals = (s0 + np.arange(SP, dtype=np.float64))[None, :]  # [1, 256]
        wrep = np.tile(w, 4)[:, None]                            # [128, 1]
        tabs = np.empty((128, 2 * SP), np.float32)
        tabs[:, 0:SP] = np.cos(wrep * svals)
        tabs[:, SP:2 * SP] = np.sin(wrep * svals)

        in_maps.append({
            "qpk": qp.astype(np.float16),
            "vpk": vpks[b],
            "wvt": wvt,
            "kgp": kgps[b],
            "gt": gt,
            "wot": wot,
            "tabs": tabs.astype(np.float16),
            "vbt": vbt,
        })
    return in_maps


def kernel(query, key, value, mask, Wq, bq, Wv, bv, Wo, bo, v_bias):
    from concourse.bass_utils import run_bass_kernel_spmd

    query = np.asarray(query, np.float32)
    key = np.asarray(key, np.float32)
    value = np.asarray(value, np.float32)
    in_maps = _host_pack(query, key, value,
                         np.asarray(Wq, np.float32), np.asarray(bq, np.float32),
                         np.asarray(Wv, np.float32), np.asarray(bv, np.float32),
                         np.asarray(Wo, np.float32), np.asarray(bo, np.float32),
                         np.asarray(v_bias, np.float32))

    if "nc" not in _cache:
        _cache["nc"] = _build_nc()
    nc = _cache["nc"]

    import os
    if int(os.environ.get("BASS_KERNEL_TRACE", "0")):
        if "antenv.axon_hooks" not in sys.modules:
            import types
            import antenv
            _mod = types.ModuleType("antenv.axon_hooks")
            _box = [None]
            _mod.set_axon_ntff_profile_hook = lambda h: _box.__setitem__(0, h)
            _mod.get_axon_ntff_profile_hook = lambda: _box[0]
            sys.modules["antenv.axon_hooks"] = _mod
            antenv.axon_hooks = _mod
            if "/root/.axon_site" not in sys.path:
                sys.path.insert(0, "/root/.axon_site")
            from trn_agent_boot.trn_boot import _ntff_profile_via_ctypes
            _mod.set_axon_ntff_profile_hook(
                _ntff_profile_via_ctypes("/opt/axon/libaxon_pjrt.so"))
    res = run_bass_kernel_spmd(
        nc, in_maps, core_ids=list(range(NC_)),
        trace=bool(int(os.environ.get("BASS_KERNEL_TRACE", "0"))))
    _cache["last_result"] = res

    out = np.empty((B, S, D), np.float32)
    for c in range(NC_):
        b, sl = c // 4, c % 4
        out[b, sl * SP:(sl + 1) * SP, :] = np.asarray(
            res.results[c]["out"], np.float32)
    return out
